# revision 11
# baseline (speedup 1.0000x reference)
"""Trainium2 Bass kernel for the BGNet MIL attention-pooling head.

Model (per reference):
  x  = LN(window_embeddings) ; h = tanh(x @ W1 + b1) ; scores = h @ w2 (+ b2)
  per-bag softmax over scores (bags = 64 contiguous windows)
  bag = sum_i w_i * window_embeddings_i                (per bag)
  y  = LN(bag) ; logits = gelu(y @ Wc1 + bc1) @ Wc2 + bc2

Distribution: data-parallel over bags — each of the 8 cores takes a
contiguous 32768-window / 512-bag slice plus replicated (host-prefolded)
parameters.

Per-core data path (all heavy tensors bf16, accumulation fp32 in PSUM):
  - one SWDGE DMA per 2048-window superblock loads x, casting fp32->bf16
  - LN stats via bn_stats/bn_aggr on the natural [win, d] layout (DVE)
  - fused (x-mu)*rstd on ScalarE (activation Identity, per-partition
    scale/bias), LN's g/b are folded into W1 on the host
  - xbar DMA-transpose to [d, win] to feed the W1 matmul (PE)
  - tanh fused with the PSUM->SBUF copy (+ folded bias) on ScalarE
  - scores via a second matmul against w2; per-bag softmax batched
    [32 bags x 64] on DVE/ScalarE
  - pooling as a block-diagonal [128,32] x [128,512] matmul over raw x
  - classifier head on-device, same folding tricks
"""

import os

os.environ.setdefault("MYCRO_LOCAL_CACHE", "1")

from contextlib import ExitStack

import ml_dtypes
import numpy as np

import concourse.bacc as bacc
import concourse.bass as bass
import concourse.mybir as mybir
import concourse.tile as tile
from concourse.bass_utils import run_bass_kernel_spmd

F32 = mybir.dt.float32
BF16 = mybir.dt.bfloat16
AFT = mybir.ActivationFunctionType

N_CORES = 8
D = 512
NCLS = 10
BAG = 64
N_WINDOWS = 262144
N_BAGS = 4096
N_LOC = N_WINDOWS // N_CORES  # 32768 windows per core
NB_LOC = N_BAGS // N_CORES    # 512 bags per core
SB_WIN = 2048                 # windows per superblock
SB_T = SB_WIN // 128          # 16 tiles of 128 windows
SB_BAGS = SB_WIN // BAG       # 32 bags
LN_EPS = 1e-5


def build_mil(tc, outs, ins, n_loc, dbg=False):
    """Emit the Tile kernel. ins/outs are dicts of DRAM APs."""
    nc = tc.nc
    n_sb = n_loc // SB_WIN
    nb = n_loc // BAG
    pbags = min(nb, 128)          # bags per partition-chunk in the classifier
    n_bchunk = (nb + pbags - 1) // pbags

    x = ins["x"]
    bag_out = outs["bag_out"]
    logits_out = outs["logits_out"]

    ctx = ExitStack()
    with ctx:
        consts = ctx.enter_context(tc.tile_pool(name="consts", bufs=1))
        rawp = ctx.enter_context(tc.tile_pool(name="rawp", bufs=4))
        xcp = ctx.enter_context(tc.tile_pool(name="xcp", bufs=3))
        xTp = ctx.enter_context(tc.tile_pool(name="xTp", bufs=2))
        thp = ctx.enter_context(tc.tile_pool(name="thp", bufs=1))
        statsp = ctx.enter_context(tc.tile_pool(name="statsp", bufs=3))
        scorep = ctx.enter_context(tc.tile_pool(name="scorep", bufs=6))
        smx = ctx.enter_context(tc.tile_pool(name="smx", bufs=2))
        scrp = ctx.enter_context(tc.tile_pool(name="scrp", bufs=2))
        php = ctx.enter_context(tc.tile_pool(name="php", bufs=4, space="PSUM"))
        psp = ctx.enter_context(tc.tile_pool(name="psp", bufs=2, space="PSUM"))
        ppoolp = ctx.enter_context(tc.tile_pool(name="ppoolp", bufs=2, space="PSUM"))

        # --- replicated params into SBUF -------------------------------
        w1g_sb = consts.tile([128, 4, D], BF16, tag="w1g")
        nc.sync.dma_start(out=w1g_sb, in_=ins["w1g"])
        v1_sb = consts.tile([128, 4], F32, tag="v1")
        nc.sync.dma_start(out=v1_sb, in_=ins["v1"])
        w2_sb = consts.tile([128, 4], BF16, tag="w2s")
        nc.sync.dma_start(out=w2_sb, in_=ins["w2s"])
        wc1g_sb = consts.tile([128, 4, D], BF16, tag="wc1g")
        nc.sync.dma_start(out=wc1g_sb, in_=ins["wc1g"])
        vc_sb = consts.tile([128, 4], F32, tag="vc")
        nc.sync.dma_start(out=vc_sb, in_=ins["vc"])
        wc2_sb = consts.tile([128, 4, NCLS], BF16, tag="wc2")
        nc.sync.dma_start(out=wc2_sb, in_=ins["wc2"])
        bc2_sb = consts.tile([1, NCLS], F32, tag="bc2")
        nc.sync.dma_start(out=bc2_sb, in_=ins["bc2"])
        eps_sb = consts.tile([128, 1], F32, tag="eps")
        nc.vector.memset(eps_sb, LN_EPS)
        ones_sb = consts.tile([1, 128], F32, tag="ones")
        nc.vector.memset(ones_sb, 1.0)
        # bag vectors accumulate here for the classifier (bag = c*128 + p)
        bagfull = consts.tile([128, n_bchunk, D], F32, tag="bagfull")

        x_r = x.rearrange("(s t w) d -> s w t d", t=SB_T, w=128)

        # ---------- software-pipelined schedule ------------------------
        # per iteration sb (emission order == engine FIFO order):
        #   W1(sb) m=0,1   | pool(sb-1) | W1(sb) m=2,3 | w2(sb)
        #   load(sb+2)     | prep(sb+1): stats+norm+transpose
        #   softmax(sb)
        # so PE never waits: pooling of sb-1 lands mid-W1(sb), and the
        # load->stats->transpose chain for sb+1 runs two stages ahead.

        def emit_load(sb):
            raw = rawp.tile([128, SB_T, D], BF16, tag="raw", name=f"raw_{sb}")
            nc.gpsimd.dma_start(out=raw, in_=x_r[sb])
            return raw

        def emit_prep(sb, raw):
            # per 4-tile group: stats (DVE) -> normalize (GpSimd) -> quarter
            # xbar transpose (SP), interleaved so transposes start early
            xcT = xTp.tile([128, SB_T * 4, 128], BF16, tag="xcT",
                           name=f"xcT_{sb}")
            for g in range(4):
                st6 = statsp.tile([128, 4, 6], F32, tag="st6",
                                  name=f"st6_{sb}_{g}")
                mv = statsp.tile([128, 4, 2], F32, tag="mv", name=f"mv_{sb}_{g}")
                for i in range(4):
                    t = 4 * g + i
                    nc.vector.bn_stats(out=st6[:, i, :], in_=raw[:, t, :])
                    nc.vector.bn_aggr(out=mv[:, i, :], in_=st6[:, i, :])
                rstd = statsp.tile([128, 4], F32, tag="rstd",
                                   name=f"rstd_{sb}_{g}")
                nc.scalar.activation(out=rstd, in_=mv[:, :, 1], func=AFT.Sqrt,
                                     bias=eps_sb, scale=1.0)
                nc.vector.reciprocal(rstd, rstd)
                xc = xcp.tile([128, 4, D], BF16, tag="xc", name=f"xc_{sb}_{g}")
                for i in range(4):
                    t = 4 * g + i
                    nc.gpsimd.tensor_scalar(out=xc[:, i, :], in0=raw[:, t, :],
                                            scalar1=mv[:, i, 0:1],
                                            scalar2=rstd[:, i:i + 1],
                                            op0=mybir.AluOpType.subtract,
                                            op1=mybir.AluOpType.mult)
                nc.sync.dma_start_transpose(
                    out=xcT[:, g * 16:(g + 1) * 16, :],
                    in_=xc.rearrange("p t d -> p (t d)"))
            return xcT.rearrange("p (t k) w -> p k t w", k=4)

        def w1_block(S, ms):
            for m in ms:
                phs = []
                for T in range(4):
                    ph = php.tile([128, D], F32, tag="ph", name=f"ph{T}")
                    phs.append(ph)
                for k in range(4):
                    for T in range(4):
                        nc.tensor.matmul(
                            phs[T],
                            lhsT=w1g_sb[:, k, m * 128:(m + 1) * 128],
                            rhs=S["xcT_k"][:, k, 4 * T:4 * T + 4, :],
                            start=(k == 0), stop=(k == 3))
                for T in range(4):
                    nc.scalar.activation(out=S["th"][T][:, m, :], in_=phs[T],
                                         func=AFT.Tanh,
                                         bias=v1_sb[:, m:m + 1], scale=1.0)

        def w2_block(S):
            scb = smx.tile([SB_BAGS, BAG], F32, tag="scb",
                           name=f"scb_{S['sb']}")
            S["scb"] = scb
            for T in range(4):
                ps = psp.tile([1, D], F32, tag="ps", name=f"ps{T}")
                for m in range(4):
                    nc.tensor.matmul(ps, lhsT=w2_sb[:, m:m + 1],
                                     rhs=S["th"][T][:, m, :],
                                     start=(m == 0), stop=(m == 3))
                sc = scorep.tile([1, D], F32, tag="sc", name=f"sc{T}")
                nc.scalar.activation(out=sc, in_=ps, func=AFT.Copy, scale=1.0)
                nc.scalar.dma_start(out=scb[8 * T:8 * (T + 1), :], in_=sc)

        def softmax_block(S):
            scb = S["scb"]
            negm = smx.tile([SB_BAGS, 1], F32, tag="negm",
                            name=f"negm_{S['sb']}")
            nc.vector.reduce_max(out=negm, in_=scb, axis=mybir.AxisListType.X,
                                 negate=True)
            wts = smx.tile([SB_BAGS, 128], BF16, tag="wts",
                           name=f"wts_{S['sb']}")
            nc.vector.memset(wts[:, BAG:], 0.0)
            dsum = smx.tile([SB_BAGS, 1], F32, tag="dsum",
                            name=f"dsum_{S['sb']}")
            nc.scalar.activation(out=wts[:, 0:BAG], in_=scb, func=AFT.Exp,
                                 bias=negm, scale=1.0, accum_out=dsum)
            rden = smx.tile([SB_BAGS, 1], F32, tag="rden",
                            name=f"rden_{S['sb']}")
            nc.vector.reciprocal(rden, dsum)
            nc.vector.tensor_scalar_mul(out=wts[:, 0:BAG], in0=wts[:, 0:BAG],
                                        scalar1=rden)
            wT = smx.tile([128, SB_BAGS], BF16, tag="wT", name=f"wT_{S['sb']}")
            nc.sync.dma_start_transpose(out=wT, in_=wts)
            wbig = smx.tile([128, SB_T, SB_BAGS], BF16, tag="wbig",
                            name=f"wbig_{S['sb']}")
            nc.vector.memset(wbig, 0.0)
            # wbig[h*64+i, t, 2t+h] = wT[i, 2t+h]: flat pos 34t+h, stride-34
            wT2 = wT.rearrange("p (t h) -> p t h", h=2)
            for h in range(2):
                nc.sync.dma_start(
                    out=wbig[h * 64:(h + 1) * 64]
                    .rearrange("p t c -> p (t c)")[:, h:h + 34 * 15 + 1:34],
                    in_=wT2[0:BAG, :, h])
            S["wbig"] = wbig

        def pool_block(S):
            sb = S["sb"]
            pp = ppoolp.tile([SB_BAGS, D], F32, tag="pp", name=f"pp_{sb}")
            for t in range(SB_T):
                nc.tensor.matmul(pp, lhsT=S["wbig"][:, t, :],
                                 rhs=S["raw"][:, t, :],
                                 start=(t == 0), stop=(t == SB_T - 1))
            scr = scrp.tile([SB_BAGS, D], F32, tag="scr", name=f"scr_{sb}")
            nc.scalar.activation(out=scr, in_=pp, func=AFT.Copy, scale=1.0)
            nc.scalar.dma_start(
                out=bag_out[sb * SB_BAGS:(sb + 1) * SB_BAGS, :], in_=scr)
            b0 = sb * SB_BAGS
            nc.scalar.dma_start(
                out=bagfull[b0 % pbags:b0 % pbags + SB_BAGS, b0 // pbags, :],
                in_=scr)

        raws = {}
        raws[0] = emit_load(0)
        if n_sb > 1:
            raws[1] = emit_load(1)
        preps = {0: emit_prep(0, raws[0])}
        prev = None
        for sb in range(n_sb):
            S = {"sb": sb, "raw": raws[sb], "xcT_k": preps.pop(sb),
                 "th": [thp.tile([128, 4, D], BF16, tag=f"th{T}",
                                 name=f"th{T}_{sb}") for T in range(4)]}
            w1_block(S, [0, 1])
            if prev is not None:
                pool_block(prev)
            w1_block(S, [2, 3])
            w2_block(S)
            if sb + 2 < n_sb:
                raws[sb + 2] = emit_load(sb + 2)
            if sb + 1 < n_sb:
                preps[sb + 1] = emit_prep(sb + 1, raws[sb + 1])
            softmax_block(S)
            if dbg and sb == 0:
                nc.gpsimd.dma_start(out=outs["dbg_scores"], in_=S["scb"])
            prev = S
        pool_block(prev)

        # ===== classifier head over all local bags =====================
        st6c = statsp.tile([128, n_bchunk, 6], F32, tag="st6c")
        mv2 = statsp.tile([128, n_bchunk, 2], F32, tag="mv2")
        for c in range(n_bchunk):
            nc.vector.bn_stats(out=st6c[:pbags, c, :], in_=bagfull[:pbags, c, :])
            nc.vector.bn_aggr(out=mv2[:pbags, c, :], in_=st6c[:pbags, c, :])
        rstd2 = statsp.tile([128, n_bchunk], F32, tag="rstd2")
        nmr2 = statsp.tile([128, n_bchunk], F32, tag="nmr2")
        nc.scalar.activation(out=rstd2[:pbags], in_=mv2[:pbags, :, 1],
                             func=AFT.Sqrt, bias=eps_sb[:pbags], scale=1.0)
        nc.vector.reciprocal(rstd2[:pbags], rstd2[:pbags])
        nc.vector.tensor_mul(nmr2[:pbags], mv2[:pbags, :, 0], rstd2[:pbags])
        nc.scalar.mul(nmr2[:pbags], nmr2[:pbags], -1.0)

        yc = consts.tile([128, n_bchunk, D], BF16, tag="yc")
        for c in range(n_bchunk):
            nc.scalar.activation(out=yc[:pbags, c, :], in_=bagfull[:pbags, c, :],
                                 func=AFT.Identity,
                                 bias=nmr2[:pbags, c:c + 1],
                                 scale=rstd2[:pbags, c:c + 1])
        ycT = consts.tile([128, n_bchunk * 4, pbags], BF16, tag="ycT")
        nc.sync.dma_start_transpose(
            out=ycT, in_=yc[:pbags].rearrange("p c d -> p (c d)"))
        ycT_k = ycT.rearrange("p (c k) w -> p k c w", k=4)

        y2T = consts.tile([128, 4, n_bchunk * pbags], BF16, tag="y2T")
        for m in range(4):
            ph = php.tile([128, n_bchunk * pbags], F32, tag="ph")
            for k in range(4):
                nc.tensor.matmul(ph, lhsT=wc1g_sb[:, k, m * 128:(m + 1) * 128],
                                 rhs=ycT_k[:, k, :, :], start=(k == 0),
                                 stop=(k == 3))
            nc.scalar.activation(out=y2T[:, m, :], in_=ph, func=AFT.Gelu,
                                 bias=vc_sb[:, m:m + 1], scale=1.0)

        lg = consts.tile([pbags, n_bchunk, NCLS], F32, tag="lg")
        for b in range(n_bchunk):
            pl = psp.tile([pbags, NCLS], F32, tag="ps")
            for k in range(4):
                nc.tensor.matmul(pl, lhsT=y2T[:, k, b * pbags:(b + 1) * pbags],
                                 rhs=wc2_sb[:, k, :], start=(k == 0), stop=False)
            nc.tensor.matmul(pl, lhsT=ones_sb[:, :pbags], rhs=bc2_sb,
                             start=False, stop=True)
            nc.scalar.activation(out=lg[:, b, :], in_=pl, func=AFT.Copy, scale=1.0)
        nc.sync.dma_start(out=logits_out.rearrange("(c p) n -> p c n", p=pbags),
                          in_=lg)


# ------------------------------------------------------------------
# host side
# ------------------------------------------------------------------

def _prep_params(inputs):
    f32 = np.float32
    W1 = np.asarray(inputs["W1"], f32)
    Wc1 = np.asarray(inputs["Wc1"], f32)
    Wc2 = np.asarray(inputs["Wc2"], f32)
    g1 = np.asarray(inputs["ln1_g"], f32)
    b1ln = np.asarray(inputs["ln1_b"], f32)
    g2 = np.asarray(inputs["ln2_g"], f32)
    b2ln = np.asarray(inputs["ln2_b"], f32)

    W1g = g1[:, None] * W1
    v1 = b1ln @ W1 + np.asarray(inputs["b1"], f32)
    Wc1g = g2[:, None] * Wc1
    vc = b2ln @ Wc1 + np.asarray(inputs["bc1"], f32)

    def chunked(a, last):
        return np.ascontiguousarray(a.reshape(4, 128, last).transpose(1, 0, 2))

    bf = ml_dtypes.bfloat16
    return {
        "w1g": chunked(W1g, D).astype(bf),
        "v1": np.ascontiguousarray(v1.reshape(4, 128).T),
        "w2s": np.ascontiguousarray(
            np.asarray(inputs["w2"], f32).reshape(4, 128).T).astype(bf),
        "wc1g": chunked(Wc1g, D).astype(bf),
        "vc": np.ascontiguousarray(vc.reshape(4, 128).T),
        "wc2": chunked(Wc2, NCLS).astype(bf),
        "bc2": np.asarray(inputs["bc2"], f32).reshape(1, NCLS),
    }


_CACHE = {}


def _get_compiled():
    if "nc" not in _CACHE:
        nc = bacc.Bacc("TRN2", target_bir_lowering=False, debug=False,
                       enable_asserts=False, num_devices=N_CORES)
        ins = {
            "x": nc.dram_tensor("x", [N_LOC, D], F32, kind="ExternalInput").ap(),
            "w1g": nc.dram_tensor("w1g", [128, 4, D], BF16,
                                  kind="ExternalInput").ap(),
            "v1": nc.dram_tensor("v1", [128, 4], F32, kind="ExternalInput").ap(),
            "w2s": nc.dram_tensor("w2s", [128, 4], BF16,
                                  kind="ExternalInput").ap(),
            "wc1g": nc.dram_tensor("wc1g", [128, 4, D], BF16,
                                   kind="ExternalInput").ap(),
            "vc": nc.dram_tensor("vc", [128, 4], F32, kind="ExternalInput").ap(),
            "wc2": nc.dram_tensor("wc2", [128, 4, NCLS], BF16,
                                  kind="ExternalInput").ap(),
            "bc2": nc.dram_tensor("bc2", [1, NCLS], F32,
                                  kind="ExternalInput").ap(),
        }
        outs = {
            "bag_out": nc.dram_tensor("bag_out", [NB_LOC, D], F32,
                                      kind="ExternalOutput").ap(),
            "logits_out": nc.dram_tensor("logits_out", [NB_LOC, NCLS], F32,
                                         kind="ExternalOutput").ap(),
        }
        with tile.TileContext(nc) as tc:
            build_mil(tc, outs, ins, N_LOC)
        nc.compile()
        _CACHE["nc"] = nc
    return _CACHE["nc"]


def make_in_maps(inputs):
    params = _prep_params(inputs)
    x = np.asarray(inputs["window_embeddings"], np.float32)
    in_maps = []
    for i in range(N_CORES):
        m = dict(params)
        m["x"] = np.ascontiguousarray(x[i * N_LOC:(i + 1) * N_LOC])
        in_maps.append(m)
    return in_maps


def kernel(**inputs):
    bag_slices = np.asarray(inputs["bag_slices"])
    starts = np.arange(N_BAGS, dtype=np.int64) * BAG
    expect = np.stack([starts, starts + BAG], axis=1)
    assert np.array_equal(bag_slices.astype(np.int64), expect), \
        "kernel assumes contiguous bags of 64 windows"

    nc = _get_compiled()
    res = run_bass_kernel_spmd(nc, make_in_maps(inputs),
                               core_ids=list(range(N_CORES)))
    logits = np.concatenate([r["logits_out"] for r in res.results], axis=0)
    bag_tensor = np.concatenate([r["bag_out"] for r in res.results], axis=0)
    return logits, bag_tensor


# revision 12
# speedup vs baseline: 3.3896x; 3.3896x over previous
"""Trainium2 Bass kernel for the BGNet MIL attention-pooling head.

Model (per reference):
  x  = LN(window_embeddings) ; h = tanh(x @ W1 + b1) ; scores = h @ w2 (+ b2)
  per-bag softmax over scores (bags = 64 contiguous windows)
  bag = sum_i w_i * window_embeddings_i                (per bag)
  y  = LN(bag) ; logits = gelu(y @ Wc1 + bc1) @ Wc2 + bc2

Distribution: data-parallel over bags — each of the 8 cores takes a
contiguous 32768-window / 512-bag slice plus replicated (host-prefolded)
parameters.

Per-core data path (all heavy tensors bf16, accumulation fp32 in PSUM):
  - one SWDGE DMA per 2048-window superblock loads x, casting fp32->bf16
  - LN stats via bn_stats/bn_aggr on the natural [win, d] layout (DVE)
  - fused (x-mu)*rstd on ScalarE (activation Identity, per-partition
    scale/bias), LN's g/b are folded into W1 on the host
  - xbar DMA-transpose to [d, win] to feed the W1 matmul (PE)
  - tanh fused with the PSUM->SBUF copy (+ folded bias) on ScalarE
  - scores via a second matmul against w2; per-bag softmax batched
    [32 bags x 64] on DVE/ScalarE
  - pooling as a block-diagonal [128,32] x [128,512] matmul over raw x
  - classifier head on-device, same folding tricks
"""

import os

os.environ.setdefault("MYCRO_LOCAL_CACHE", "1")

from contextlib import ExitStack

import ml_dtypes
import numpy as np

import concourse.bacc as bacc
import concourse.bass as bass
import concourse.mybir as mybir
import concourse.tile as tile
from concourse.bass_utils import run_bass_kernel_spmd

F32 = mybir.dt.float32
BF16 = mybir.dt.bfloat16
AFT = mybir.ActivationFunctionType

N_CORES = 8
D = 512
NCLS = 10
BAG = 64
N_WINDOWS = 262144
N_BAGS = 4096
N_LOC = N_WINDOWS // N_CORES  # 32768 windows per core
NB_LOC = N_BAGS // N_CORES    # 512 bags per core
SB_WIN = 2048                 # windows per superblock
SB_T = SB_WIN // 128          # 16 tiles of 128 windows
SB_BAGS = SB_WIN // BAG       # 32 bags
LN_EPS = 1e-5


def build_mil(tc, outs, ins, n_loc, dbg=False):
    """Emit the Tile kernel. ins/outs are dicts of DRAM APs."""
    nc = tc.nc
    n_sb = n_loc // SB_WIN
    nb = n_loc // BAG
    pbags = min(nb, 128)          # bags per partition-chunk in the classifier
    n_bchunk = (nb + pbags - 1) // pbags

    x = ins["x"]
    bag_out = outs["bag_out"]
    logits_out = outs["logits_out"]

    ctx = ExitStack()
    with ctx:
        consts = ctx.enter_context(tc.tile_pool(name="consts", bufs=1))
        rawp = ctx.enter_context(tc.tile_pool(name="rawp", bufs=4))
        xcp = ctx.enter_context(tc.tile_pool(name="xcp", bufs=3))
        xTp = ctx.enter_context(tc.tile_pool(name="xTp", bufs=2))
        thp = ctx.enter_context(tc.tile_pool(name="thp", bufs=1))
        statsp = ctx.enter_context(tc.tile_pool(name="statsp", bufs=3))
        scorep = ctx.enter_context(tc.tile_pool(name="scorep", bufs=6))
        smx = ctx.enter_context(tc.tile_pool(name="smx", bufs=2))
        scrp = ctx.enter_context(tc.tile_pool(name="scrp", bufs=2))
        php = ctx.enter_context(tc.tile_pool(name="php", bufs=4, space="PSUM"))
        psp = ctx.enter_context(tc.tile_pool(name="psp", bufs=2, space="PSUM"))
        ppoolp = ctx.enter_context(tc.tile_pool(name="ppoolp", bufs=2, space="PSUM"))

        # --- replicated params into SBUF -------------------------------
        w1g_sb = consts.tile([128, 4, D], BF16, tag="w1g")
        nc.sync.dma_start(out=w1g_sb, in_=ins["w1g"])
        v1_sb = consts.tile([128, 4], F32, tag="v1")
        nc.sync.dma_start(out=v1_sb, in_=ins["v1"])
        w2_sb = consts.tile([128, 4], BF16, tag="w2s")
        nc.sync.dma_start(out=w2_sb, in_=ins["w2s"])
        wc1g_sb = consts.tile([128, 4, D], BF16, tag="wc1g")
        nc.sync.dma_start(out=wc1g_sb, in_=ins["wc1g"])
        vc_sb = consts.tile([128, 4], F32, tag="vc")
        nc.sync.dma_start(out=vc_sb, in_=ins["vc"])
        wc2_sb = consts.tile([128, 4, NCLS], BF16, tag="wc2")
        nc.sync.dma_start(out=wc2_sb, in_=ins["wc2"])
        bc2_sb = consts.tile([1, NCLS], F32, tag="bc2")
        nc.sync.dma_start(out=bc2_sb, in_=ins["bc2"])
        eps_sb = consts.tile([128, 1], F32, tag="eps")
        nc.vector.memset(eps_sb, LN_EPS)
        ones_sb = consts.tile([1, 128], F32, tag="ones")
        nc.vector.memset(ones_sb, 1.0)
        # bag vectors accumulate here for the classifier (bag = c*128 + p)
        bagfull = consts.tile([128, n_bchunk, D], F32, tag="bagfull")

        x_r = x.rearrange("(s t w) d -> s w t d", t=SB_T, w=128)

        # ---------- software-pipelined schedule ------------------------
        # per iteration sb (emission order == engine FIFO order):
        #   W1(sb) m=0,1   | pool(sb-1) | W1(sb) m=2,3 | w2(sb)
        #   load(sb+2)     | prep(sb+1): stats+norm+transpose
        #   softmax(sb)
        # so PE never waits: pooling of sb-1 lands mid-W1(sb), and the
        # load->stats->transpose chain for sb+1 runs two stages ahead.

        def emit_load(sb):
            raw = rawp.tile([128, SB_T, D], BF16, tag="raw", name=f"raw_{sb}")
            nc.gpsimd.dma_start(out=raw, in_=x_r[sb])
            return raw

        def emit_prep(sb, raw):
            # per 4-tile group: stats (DVE) -> normalize (GpSimd) -> quarter
            # xbar transpose (SP), interleaved so transposes start early
            xcT = xTp.tile([128, SB_T * 4, 128], BF16, tag="xcT",
                           name=f"xcT_{sb}")
            for g in range(4):
                st6 = statsp.tile([128, 4, 6], F32, tag="st6",
                                  name=f"st6_{sb}_{g}")
                mv = statsp.tile([128, 4, 2], F32, tag="mv", name=f"mv_{sb}_{g}")
                for i in range(4):
                    t = 4 * g + i
                    nc.vector.bn_stats(out=st6[:, i, :], in_=raw[:, t, :])
                    nc.vector.bn_aggr(out=mv[:, i, :], in_=st6[:, i, :])
                rstd = statsp.tile([128, 4], F32, tag="rstd",
                                   name=f"rstd_{sb}_{g}")
                nc.scalar.activation(out=rstd, in_=mv[:, :, 1], func=AFT.Sqrt,
                                     bias=eps_sb, scale=1.0)
                nc.vector.reciprocal(rstd, rstd)
                xc = xcp.tile([128, 4, D], BF16, tag="xc", name=f"xc_{sb}_{g}")
                for i in range(4):
                    t = 4 * g + i
                    nc.vector.tensor_scalar(out=xc[:, i, :], in0=raw[:, t, :],
                                            scalar1=mv[:, i, 0:1],
                                            scalar2=rstd[:, i:i + 1],
                                            op0=mybir.AluOpType.subtract,
                                            op1=mybir.AluOpType.mult)
                nc.sync.dma_start_transpose(
                    out=xcT[:, g * 16:(g + 1) * 16, :],
                    in_=xc.rearrange("p t d -> p (t d)"))
            return xcT.rearrange("p (t k) w -> p k t w", k=4)

        def w1_block(S, ms):
            for m in ms:
                phs = []
                for T in range(4):
                    ph = php.tile([128, D], F32, tag="ph", name=f"ph{T}")
                    phs.append(ph)
                for k in range(4):
                    for T in range(4):
                        nc.tensor.matmul(
                            phs[T],
                            lhsT=w1g_sb[:, k, m * 128:(m + 1) * 128],
                            rhs=S["xcT_k"][:, k, 4 * T:4 * T + 4, :],
                            start=(k == 0), stop=(k == 3))
                for T in range(4):
                    nc.scalar.activation(out=S["th"][T][:, m, :], in_=phs[T],
                                         func=AFT.Tanh,
                                         bias=v1_sb[:, m:m + 1], scale=1.0)

        def w2_block(S):
            scb = smx.tile([SB_BAGS, BAG], F32, tag="scb",
                           name=f"scb_{S['sb']}")
            S["scb"] = scb
            for T in range(4):
                ps = psp.tile([1, D], F32, tag="ps", name=f"ps{T}")
                for m in range(4):
                    nc.tensor.matmul(ps, lhsT=w2_sb[:, m:m + 1],
                                     rhs=S["th"][T][:, m, :],
                                     start=(m == 0), stop=(m == 3))
                sc = scorep.tile([1, D], F32, tag="sc", name=f"sc{T}")
                nc.scalar.activation(out=sc, in_=ps, func=AFT.Copy, scale=1.0)
                nc.scalar.dma_start(out=scb[8 * T:8 * (T + 1), :], in_=sc)

        def softmax_block(S):
            scb = S["scb"]
            negm = smx.tile([SB_BAGS, 1], F32, tag="negm",
                            name=f"negm_{S['sb']}")
            nc.vector.reduce_max(out=negm, in_=scb, axis=mybir.AxisListType.X,
                                 negate=True)
            wts = smx.tile([SB_BAGS, 128], BF16, tag="wts",
                           name=f"wts_{S['sb']}")
            nc.vector.memset(wts[:, BAG:], 0.0)
            dsum = smx.tile([SB_BAGS, 1], F32, tag="dsum",
                            name=f"dsum_{S['sb']}")
            nc.scalar.activation(out=wts[:, 0:BAG], in_=scb, func=AFT.Exp,
                                 bias=negm, scale=1.0, accum_out=dsum)
            rden = smx.tile([SB_BAGS, 1], F32, tag="rden",
                            name=f"rden_{S['sb']}")
            nc.vector.reciprocal(rden, dsum)
            nc.vector.tensor_scalar_mul(out=wts[:, 0:BAG], in0=wts[:, 0:BAG],
                                        scalar1=rden)
            wT = smx.tile([128, SB_BAGS], BF16, tag="wT", name=f"wT_{S['sb']}")
            nc.sync.dma_start_transpose(out=wT, in_=wts)
            wbig = smx.tile([128, SB_T, SB_BAGS], BF16, tag="wbig",
                            name=f"wbig_{S['sb']}")
            nc.vector.memset(wbig, 0.0)
            # wbig[h*64+i, t, 2t+h] = wT[i, 2t+h]: flat pos 34t+h, stride-34
            wT2 = wT.rearrange("p (t h) -> p t h", h=2)
            for h in range(2):
                nc.sync.dma_start(
                    out=wbig[h * 64:(h + 1) * 64]
                    .rearrange("p t c -> p (t c)")[:, h:h + 34 * 15 + 1:34],
                    in_=wT2[0:BAG, :, h])
            S["wbig"] = wbig

        def pool_block(S):
            sb = S["sb"]
            pp = ppoolp.tile([SB_BAGS, D], F32, tag="pp", name=f"pp_{sb}")
            for t in range(SB_T):
                nc.tensor.matmul(pp, lhsT=S["wbig"][:, t, :],
                                 rhs=S["raw"][:, t, :],
                                 start=(t == 0), stop=(t == SB_T - 1))
            scr = scrp.tile([SB_BAGS, D], F32, tag="scr", name=f"scr_{sb}")
            nc.scalar.activation(out=scr, in_=pp, func=AFT.Copy, scale=1.0)
            nc.scalar.dma_start(
                out=bag_out[sb * SB_BAGS:(sb + 1) * SB_BAGS, :], in_=scr)
            b0 = sb * SB_BAGS
            nc.scalar.dma_start(
                out=bagfull[b0 % pbags:b0 % pbags + SB_BAGS, b0 // pbags, :],
                in_=scr)

        raws = {}
        raws[0] = emit_load(0)
        if n_sb > 1:
            raws[1] = emit_load(1)
        preps = {0: emit_prep(0, raws[0])}
        prev = None
        for sb in range(n_sb):
            S = {"sb": sb, "raw": raws[sb], "xcT_k": preps.pop(sb),
                 "th": [thp.tile([128, 4, D], BF16, tag=f"th{T}",
                                 name=f"th{T}_{sb}") for T in range(4)]}
            w1_block(S, [0, 1])
            if prev is not None:
                pool_block(prev)
            w1_block(S, [2, 3])
            w2_block(S)
            if sb + 2 < n_sb:
                raws[sb + 2] = emit_load(sb + 2)
            if sb + 1 < n_sb:
                preps[sb + 1] = emit_prep(sb + 1, raws[sb + 1])
            softmax_block(S)
            if dbg and sb == 0:
                nc.gpsimd.dma_start(out=outs["dbg_scores"], in_=S["scb"])
            prev = S
        pool_block(prev)

        # ===== classifier head over all local bags =====================
        st6c = statsp.tile([128, n_bchunk, 6], F32, tag="st6c")
        mv2 = statsp.tile([128, n_bchunk, 2], F32, tag="mv2")
        for c in range(n_bchunk):
            nc.vector.bn_stats(out=st6c[:pbags, c, :], in_=bagfull[:pbags, c, :])
            nc.vector.bn_aggr(out=mv2[:pbags, c, :], in_=st6c[:pbags, c, :])
        rstd2 = statsp.tile([128, n_bchunk], F32, tag="rstd2")
        nmr2 = statsp.tile([128, n_bchunk], F32, tag="nmr2")
        nc.scalar.activation(out=rstd2[:pbags], in_=mv2[:pbags, :, 1],
                             func=AFT.Sqrt, bias=eps_sb[:pbags], scale=1.0)
        nc.vector.reciprocal(rstd2[:pbags], rstd2[:pbags])
        nc.vector.tensor_mul(nmr2[:pbags], mv2[:pbags, :, 0], rstd2[:pbags])
        nc.scalar.mul(nmr2[:pbags], nmr2[:pbags], -1.0)

        yc = consts.tile([128, n_bchunk, D], BF16, tag="yc")
        for c in range(n_bchunk):
            nc.scalar.activation(out=yc[:pbags, c, :], in_=bagfull[:pbags, c, :],
                                 func=AFT.Identity,
                                 bias=nmr2[:pbags, c:c + 1],
                                 scale=rstd2[:pbags, c:c + 1])
        ycT = consts.tile([128, n_bchunk * 4, pbags], BF16, tag="ycT")
        nc.sync.dma_start_transpose(
            out=ycT, in_=yc[:pbags].rearrange("p c d -> p (c d)"))
        ycT_k = ycT.rearrange("p (c k) w -> p k c w", k=4)

        y2T = consts.tile([128, 4, n_bchunk * pbags], BF16, tag="y2T")
        for m in range(4):
            ph = php.tile([128, n_bchunk * pbags], F32, tag="ph")
            for k in range(4):
                nc.tensor.matmul(ph, lhsT=wc1g_sb[:, k, m * 128:(m + 1) * 128],
                                 rhs=ycT_k[:, k, :, :], start=(k == 0),
                                 stop=(k == 3))
            nc.scalar.activation(out=y2T[:, m, :], in_=ph, func=AFT.Gelu,
                                 bias=vc_sb[:, m:m + 1], scale=1.0)

        lg = consts.tile([pbags, n_bchunk, NCLS], F32, tag="lg")
        for b in range(n_bchunk):
            pl = psp.tile([pbags, NCLS], F32, tag="ps")
            for k in range(4):
                nc.tensor.matmul(pl, lhsT=y2T[:, k, b * pbags:(b + 1) * pbags],
                                 rhs=wc2_sb[:, k, :], start=(k == 0), stop=False)
            nc.tensor.matmul(pl, lhsT=ones_sb[:, :pbags], rhs=bc2_sb,
                             start=False, stop=True)
            nc.scalar.activation(out=lg[:, b, :], in_=pl, func=AFT.Copy, scale=1.0)
        nc.sync.dma_start(out=logits_out.rearrange("(c p) n -> p c n", p=pbags),
                          in_=lg)


# ------------------------------------------------------------------
# host side
# ------------------------------------------------------------------

def _prep_params(inputs):
    f32 = np.float32
    W1 = np.asarray(inputs["W1"], f32)
    Wc1 = np.asarray(inputs["Wc1"], f32)
    Wc2 = np.asarray(inputs["Wc2"], f32)
    g1 = np.asarray(inputs["ln1_g"], f32)
    b1ln = np.asarray(inputs["ln1_b"], f32)
    g2 = np.asarray(inputs["ln2_g"], f32)
    b2ln = np.asarray(inputs["ln2_b"], f32)

    W1g = g1[:, None] * W1
    v1 = b1ln @ W1 + np.asarray(inputs["b1"], f32)
    Wc1g = g2[:, None] * Wc1
    vc = b2ln @ Wc1 + np.asarray(inputs["bc1"], f32)

    def chunked(a, last):
        return np.ascontiguousarray(a.reshape(4, 128, last).transpose(1, 0, 2))

    bf = ml_dtypes.bfloat16
    return {
        "w1g": chunked(W1g, D).astype(bf),
        "v1": np.ascontiguousarray(v1.reshape(4, 128).T),
        "w2s": np.ascontiguousarray(
            np.asarray(inputs["w2"], f32).reshape(4, 128).T).astype(bf),
        "wc1g": chunked(Wc1g, D).astype(bf),
        "vc": np.ascontiguousarray(vc.reshape(4, 128).T),
        "wc2": chunked(Wc2, NCLS).astype(bf),
        "bc2": np.asarray(inputs["bc2"], f32).reshape(1, NCLS),
    }


_CACHE = {}


def _get_compiled():
    if "nc" not in _CACHE:
        nc = bacc.Bacc("TRN2", target_bir_lowering=False, debug=False,
                       enable_asserts=False, num_devices=N_CORES)
        ins = {
            "x": nc.dram_tensor("x", [N_LOC, D], F32, kind="ExternalInput").ap(),
            "w1g": nc.dram_tensor("w1g", [128, 4, D], BF16,
                                  kind="ExternalInput").ap(),
            "v1": nc.dram_tensor("v1", [128, 4], F32, kind="ExternalInput").ap(),
            "w2s": nc.dram_tensor("w2s", [128, 4], BF16,
                                  kind="ExternalInput").ap(),
            "wc1g": nc.dram_tensor("wc1g", [128, 4, D], BF16,
                                   kind="ExternalInput").ap(),
            "vc": nc.dram_tensor("vc", [128, 4], F32, kind="ExternalInput").ap(),
            "wc2": nc.dram_tensor("wc2", [128, 4, NCLS], BF16,
                                  kind="ExternalInput").ap(),
            "bc2": nc.dram_tensor("bc2", [1, NCLS], F32,
                                  kind="ExternalInput").ap(),
        }
        outs = {
            "bag_out": nc.dram_tensor("bag_out", [NB_LOC, D], F32,
                                      kind="ExternalOutput").ap(),
            "logits_out": nc.dram_tensor("logits_out", [NB_LOC, NCLS], F32,
                                         kind="ExternalOutput").ap(),
        }
        with tile.TileContext(nc) as tc:
            build_mil(tc, outs, ins, N_LOC)
        nc.compile()
        _CACHE["nc"] = nc
    return _CACHE["nc"]


def make_in_maps(inputs):
    params = _prep_params(inputs)
    x = np.asarray(inputs["window_embeddings"], np.float32)
    in_maps = []
    for i in range(N_CORES):
        m = dict(params)
        m["x"] = np.ascontiguousarray(x[i * N_LOC:(i + 1) * N_LOC])
        in_maps.append(m)
    return in_maps


def kernel(**inputs):
    bag_slices = np.asarray(inputs["bag_slices"])
    starts = np.arange(N_BAGS, dtype=np.int64) * BAG
    expect = np.stack([starts, starts + BAG], axis=1)
    assert np.array_equal(bag_slices.astype(np.int64), expect), \
        "kernel assumes contiguous bags of 64 windows"

    nc = _get_compiled()
    res = run_bass_kernel_spmd(nc, make_in_maps(inputs),
                               core_ids=list(range(N_CORES)))
    logits = np.concatenate([r["logits_out"] for r in res.results], axis=0)
    bag_tensor = np.concatenate([r["bag_out"] for r in res.results], axis=0)
    return logits, bag_tensor


# revision 13
# speedup vs baseline: 3.6695x; 1.0826x over previous
"""Trainium2 Bass kernel for the BGNet MIL attention-pooling head.

Model (per reference):
  x  = LN(window_embeddings) ; h = tanh(x @ W1 + b1) ; scores = h @ w2 (+ b2)
  per-bag softmax over scores (bags = 64 contiguous windows)
  bag = sum_i w_i * window_embeddings_i                (per bag)
  y  = LN(bag) ; logits = gelu(y @ Wc1 + bc1) @ Wc2 + bc2

Distribution: data-parallel over bags — each of the 8 cores takes a
contiguous 32768-window / 512-bag slice plus replicated (host-prefolded)
parameters.

Per-core data path (all heavy tensors bf16, accumulation fp32 in PSUM):
  - one SWDGE DMA per 2048-window superblock loads x, casting fp32->bf16
  - LN stats via bn_stats/bn_aggr on the natural [win, d] layout (DVE)
  - fused (x-mu)*rstd on ScalarE (activation Identity, per-partition
    scale/bias), LN's g/b are folded into W1 on the host
  - xbar DMA-transpose to [d, win] to feed the W1 matmul (PE)
  - tanh fused with the PSUM->SBUF copy (+ folded bias) on ScalarE
  - scores via a second matmul against w2; per-bag softmax batched
    [32 bags x 64] on DVE/ScalarE
  - pooling as a block-diagonal [128,32] x [128,512] matmul over raw x
  - classifier head on-device, same folding tricks
"""

import os

os.environ.setdefault("MYCRO_LOCAL_CACHE", "1")

from contextlib import ExitStack

import ml_dtypes
import numpy as np

import concourse.bacc as bacc
import concourse.bass as bass
import concourse.mybir as mybir
import concourse.tile as tile
from concourse.bass_utils import run_bass_kernel_spmd

F32 = mybir.dt.float32
BF16 = mybir.dt.bfloat16
AFT = mybir.ActivationFunctionType

N_CORES = 8
D = 512
NCLS = 10
BAG = 64
N_WINDOWS = 262144
N_BAGS = 4096
N_LOC = N_WINDOWS // N_CORES  # 32768 windows per core
NB_LOC = N_BAGS // N_CORES    # 512 bags per core
SB_WIN = 2048                 # windows per superblock
SB_T = SB_WIN // 128          # 16 tiles of 128 windows
SB_BAGS = SB_WIN // BAG       # 32 bags
LN_EPS = 1e-5


def build_mil(tc, outs, ins, n_loc, dbg=False):
    """Emit the Tile kernel. ins/outs are dicts of DRAM APs."""
    nc = tc.nc
    n_sb = n_loc // SB_WIN
    nb = n_loc // BAG
    pbags = min(nb, 128)          # bags per partition-chunk in the classifier
    n_bchunk = (nb + pbags - 1) // pbags

    x = ins["x"]
    bag_out = outs["bag_out"]
    logits_out = outs["logits_out"]

    ctx = ExitStack()
    with ctx:
        consts = ctx.enter_context(tc.tile_pool(name="consts", bufs=1))
        rawp = ctx.enter_context(tc.tile_pool(name="rawp", bufs=4))
        xcp = ctx.enter_context(tc.tile_pool(name="xcp", bufs=3))
        xTp = ctx.enter_context(tc.tile_pool(name="xTp", bufs=3))
        thp = ctx.enter_context(tc.tile_pool(name="thp", bufs=1))
        statsp = ctx.enter_context(tc.tile_pool(name="statsp", bufs=3))
        scorep = ctx.enter_context(tc.tile_pool(name="scorep", bufs=4))
        smx = ctx.enter_context(tc.tile_pool(name="smx", bufs=2))
        scrp = ctx.enter_context(tc.tile_pool(name="scrp", bufs=2))
        php = ctx.enter_context(tc.tile_pool(name="php", bufs=4, space="PSUM"))
        psp = ctx.enter_context(tc.tile_pool(name="psp", bufs=2, space="PSUM"))
        ppoolp = ctx.enter_context(tc.tile_pool(name="ppoolp", bufs=2, space="PSUM"))

        # --- replicated params into SBUF -------------------------------
        w1g_sb = consts.tile([128, 4, D], BF16, tag="w1g")
        nc.sync.dma_start(out=w1g_sb, in_=ins["w1g"])
        v1_sb = consts.tile([128, 4], F32, tag="v1")
        nc.sync.dma_start(out=v1_sb, in_=ins["v1"])
        w2_sb = consts.tile([128, 4], BF16, tag="w2s")
        nc.sync.dma_start(out=w2_sb, in_=ins["w2s"])
        wc1g_sb = consts.tile([128, 4, D], BF16, tag="wc1g")
        nc.sync.dma_start(out=wc1g_sb, in_=ins["wc1g"])
        vc_sb = consts.tile([128, 4], F32, tag="vc")
        nc.sync.dma_start(out=vc_sb, in_=ins["vc"])
        wc2_sb = consts.tile([128, 4, NCLS], BF16, tag="wc2")
        nc.sync.dma_start(out=wc2_sb, in_=ins["wc2"])
        bc2_sb = consts.tile([1, NCLS], F32, tag="bc2")
        nc.sync.dma_start(out=bc2_sb, in_=ins["bc2"])
        eps_sb = consts.tile([128, 1], F32, tag="eps")
        nc.vector.memset(eps_sb, LN_EPS)
        ones_sb = consts.tile([1, 128], F32, tag="ones")
        nc.vector.memset(ones_sb, 1.0)
        # bag vectors accumulate here for the classifier (bag = c*128 + p)
        bagfull = consts.tile([128, n_bchunk, D], F32, tag="bagfull")

        x_r = x.rearrange("(s t w) d -> s w t d", t=SB_T, w=128)

        # ---------- software-pipelined schedule ------------------------
        # per iteration sb (emission order == engine FIFO order):
        #   W1(sb) m=0,1   | pool(sb-1) | W1(sb) m=2,3 | w2(sb)
        #   load(sb+2)     | prep(sb+1): stats+norm+transpose
        #   softmax(sb)
        # so PE never waits: pooling of sb-1 lands mid-W1(sb), and the
        # load->stats->transpose chain for sb+1 runs two stages ahead.

        def emit_load(sb):
            raw = rawp.tile([128, SB_T, D], BF16, tag="raw", name=f"raw_{sb}")
            nc.gpsimd.dma_start(out=raw, in_=x_r[sb])
            return raw

        def emit_prep(sb, raw):
            # per 4-tile group: stats (DVE) -> normalize (GpSimd) -> quarter
            # xbar transpose (SP), interleaved so transposes start early
            xcT = xTp.tile([128, SB_T * 4, 128], BF16, tag="xcT",
                           name=f"xcT_{sb}")
            for g in range(4):
                st6 = statsp.tile([128, 4, 6], F32, tag="st6",
                                  name=f"st6_{sb}_{g}")
                mv = statsp.tile([128, 4, 2], F32, tag="mv", name=f"mv_{sb}_{g}")
                for i in range(4):
                    t = 4 * g + i
                    nc.vector.bn_stats(out=st6[:, i, :], in_=raw[:, t, :])
                    nc.vector.bn_aggr(out=mv[:, i, :], in_=st6[:, i, :])
                rstd = statsp.tile([128, 4], F32, tag="rstd",
                                   name=f"rstd_{sb}_{g}")
                nc.scalar.activation(out=rstd, in_=mv[:, :, 1], func=AFT.Sqrt,
                                     bias=eps_sb, scale=1.0)
                nc.vector.reciprocal(rstd, rstd)
                xc = xcp.tile([128, 4, D], BF16, tag="xc", name=f"xc_{sb}_{g}")
                for i in range(4):
                    t = 4 * g + i
                    nc.vector.tensor_scalar(out=xc[:, i, :], in0=raw[:, t, :],
                                            scalar1=mv[:, i, 0:1],
                                            scalar2=rstd[:, i:i + 1],
                                            op0=mybir.AluOpType.subtract,
                                            op1=mybir.AluOpType.mult)
                nc.sync.dma_start_transpose(
                    out=xcT[:, g * 16:(g + 1) * 16, :],
                    in_=xc.rearrange("p t d -> p (t d)"))
            return xcT.rearrange("p (t k) w -> p k t w", k=4)

        def w1_block(S, ms):
            for m in ms:
                phs = []
                for T in range(4):
                    ph = php.tile([128, D], F32, tag="ph", name=f"ph{T}")
                    phs.append(ph)
                for k in range(4):
                    for T in range(4):
                        nc.tensor.matmul(
                            phs[T],
                            lhsT=w1g_sb[:, k, m * 128:(m + 1) * 128],
                            rhs=S["xcT_k"][:, k, 4 * T:4 * T + 4, :],
                            start=(k == 0), stop=(k == 3))
                for T in range(4):
                    nc.scalar.activation(out=S["th"][T][:, m, :], in_=phs[T],
                                         func=AFT.Tanh,
                                         bias=v1_sb[:, m:m + 1], scale=1.0)

        def w2_block(S):
            scb = smx.tile([SB_BAGS, BAG], F32, tag="scb",
                           name=f"scb_{S['sb']}")
            S["scb"] = scb
            for T in range(4):
                ps = psp.tile([1, D], F32, tag="ps", name=f"ps{T}")
                for m in range(4):
                    nc.tensor.matmul(ps, lhsT=w2_sb[:, m:m + 1],
                                     rhs=S["th"][T][:, m, :],
                                     start=(m == 0), stop=(m == 3))
                sc = scorep.tile([1, D], F32, tag="sc", name=f"sc{T}")
                nc.scalar.activation(out=sc, in_=ps, func=AFT.Copy, scale=1.0)
                nc.scalar.dma_start(out=scb[8 * T:8 * (T + 1), :], in_=sc)

        def softmax_block(S):
            scb = S["scb"]
            negm = smx.tile([SB_BAGS, 1], F32, tag="negm",
                            name=f"negm_{S['sb']}")
            nc.vector.reduce_max(out=negm, in_=scb, axis=mybir.AxisListType.X,
                                 negate=True)
            wts = smx.tile([SB_BAGS, 128], BF16, tag="wts",
                           name=f"wts_{S['sb']}")
            nc.vector.memset(wts[:, BAG:], 0.0)
            dsum = smx.tile([SB_BAGS, 1], F32, tag="dsum",
                            name=f"dsum_{S['sb']}")
            nc.scalar.activation(out=wts[:, 0:BAG], in_=scb, func=AFT.Exp,
                                 bias=negm, scale=1.0, accum_out=dsum)
            rden = smx.tile([SB_BAGS, 1], F32, tag="rden",
                            name=f"rden_{S['sb']}")
            nc.vector.reciprocal(rden, dsum)
            nc.vector.tensor_scalar_mul(out=wts[:, 0:BAG], in0=wts[:, 0:BAG],
                                        scalar1=rden)
            wT = smx.tile([128, SB_BAGS], BF16, tag="wT", name=f"wT_{S['sb']}")
            nc.sync.dma_start_transpose(out=wT, in_=wts)
            wbig = smx.tile([128, SB_T, SB_BAGS], BF16, tag="wbig",
                            name=f"wbig_{S['sb']}")
            nc.vector.memset(wbig, 0.0)
            # wbig[h*64+i, t, 2t+h] = wT[i, 2t+h]: flat pos 34t+h, stride-34
            wT2 = wT.rearrange("p (t h) -> p t h", h=2)
            for h in range(2):
                nc.sync.dma_start(
                    out=wbig[h * 64:(h + 1) * 64]
                    .rearrange("p t c -> p (t c)")[:, h:h + 34 * 15 + 1:34],
                    in_=wT2[0:BAG, :, h])
            S["wbig"] = wbig

        def pool_block(S):
            sb = S["sb"]
            pp = ppoolp.tile([SB_BAGS, D], F32, tag="pp", name=f"pp_{sb}")
            for t in range(SB_T):
                nc.tensor.matmul(pp, lhsT=S["wbig"][:, t, :],
                                 rhs=S["raw"][:, t, :],
                                 start=(t == 0), stop=(t == SB_T - 1))
            scr = scrp.tile([SB_BAGS, D], F32, tag="scr", name=f"scr_{sb}")
            nc.vector.tensor_copy(out=scr, in_=pp)
            nc.scalar.dma_start(
                out=bag_out[sb * SB_BAGS:(sb + 1) * SB_BAGS, :], in_=scr)
            b0 = sb * SB_BAGS
            nc.scalar.dma_start(
                out=bagfull[b0 % pbags:b0 % pbags + SB_BAGS, b0 // pbags, :],
                in_=scr)

        raws = {}
        preps = {}
        for i in range(min(3, n_sb)):
            raws[i] = emit_load(i)
        for i in range(min(2, n_sb)):
            preps[i] = emit_prep(i, raws[i])
        prev = None
        for sb in range(n_sb):
            S = {"sb": sb, "raw": raws[sb], "xcT_k": preps.pop(sb),
                 "th": [thp.tile([128, 4, D], BF16, tag=f"th{T}",
                                 name=f"th{T}_{sb}") for T in range(4)]}
            w1_block(S, [0, 1, 2, 3])
            w2_block(S)
            if sb + 2 < n_sb:
                preps[sb + 2] = emit_prep(sb + 2, raws[sb + 2])
            softmax_block(S)
            if dbg and sb == 0:
                nc.gpsimd.dma_start(out=outs["dbg_scores"], in_=S["scb"])
            if prev is not None:
                pool_block(prev)
            if sb + 3 < n_sb:
                raws[sb + 3] = emit_load(sb + 3)
            prev = S
        pool_block(prev)

        # ===== classifier head over all local bags =====================
        st6c = statsp.tile([128, n_bchunk, 6], F32, tag="st6c")
        mv2 = statsp.tile([128, n_bchunk, 2], F32, tag="mv2")
        for c in range(n_bchunk):
            nc.vector.bn_stats(out=st6c[:pbags, c, :], in_=bagfull[:pbags, c, :])
            nc.vector.bn_aggr(out=mv2[:pbags, c, :], in_=st6c[:pbags, c, :])
        rstd2 = statsp.tile([128, n_bchunk], F32, tag="rstd2")
        nmr2 = statsp.tile([128, n_bchunk], F32, tag="nmr2")
        nc.scalar.activation(out=rstd2[:pbags], in_=mv2[:pbags, :, 1],
                             func=AFT.Sqrt, bias=eps_sb[:pbags], scale=1.0)
        nc.vector.reciprocal(rstd2[:pbags], rstd2[:pbags])
        nc.vector.tensor_mul(nmr2[:pbags], mv2[:pbags, :, 0], rstd2[:pbags])
        nc.scalar.mul(nmr2[:pbags], nmr2[:pbags], -1.0)

        yc = xcp.tile([128, n_bchunk, D], BF16, tag="xc")
        for c in range(n_bchunk):
            nc.scalar.activation(out=yc[:pbags, c, :], in_=bagfull[:pbags, c, :],
                                 func=AFT.Identity,
                                 bias=nmr2[:pbags, c:c + 1],
                                 scale=rstd2[:pbags, c:c + 1])
        ycT = xTp.tile([128, n_bchunk * 4, pbags], BF16, tag="xcT")
        nc.sync.dma_start_transpose(
            out=ycT, in_=yc[:pbags].rearrange("p c d -> p (c d)"))
        ycT_k = ycT.rearrange("p (c k) w -> p k c w", k=4)

        y2T = thp.tile([128, 4, n_bchunk * pbags], BF16, tag="th0")
        for m in range(4):
            ph = php.tile([128, n_bchunk * pbags], F32, tag="ph")
            for k in range(4):
                nc.tensor.matmul(ph, lhsT=wc1g_sb[:, k, m * 128:(m + 1) * 128],
                                 rhs=ycT_k[:, k, :, :], start=(k == 0),
                                 stop=(k == 3))
            nc.scalar.activation(out=y2T[:, m, :], in_=ph, func=AFT.Gelu,
                                 bias=vc_sb[:, m:m + 1], scale=1.0)

        lg = consts.tile([pbags, n_bchunk, NCLS], F32, tag="lg")
        for b in range(n_bchunk):
            pl = psp.tile([pbags, NCLS], F32, tag="ps")
            for k in range(4):
                nc.tensor.matmul(pl, lhsT=y2T[:, k, b * pbags:(b + 1) * pbags],
                                 rhs=wc2_sb[:, k, :], start=(k == 0), stop=False)
            nc.tensor.matmul(pl, lhsT=ones_sb[:, :pbags], rhs=bc2_sb,
                             start=False, stop=True)
            nc.scalar.activation(out=lg[:, b, :], in_=pl, func=AFT.Copy, scale=1.0)
        nc.sync.dma_start(out=logits_out.rearrange("(c p) n -> p c n", p=pbags),
                          in_=lg)


# ------------------------------------------------------------------
# host side
# ------------------------------------------------------------------

def _prep_params(inputs):
    f32 = np.float32
    W1 = np.asarray(inputs["W1"], f32)
    Wc1 = np.asarray(inputs["Wc1"], f32)
    Wc2 = np.asarray(inputs["Wc2"], f32)
    g1 = np.asarray(inputs["ln1_g"], f32)
    b1ln = np.asarray(inputs["ln1_b"], f32)
    g2 = np.asarray(inputs["ln2_g"], f32)
    b2ln = np.asarray(inputs["ln2_b"], f32)

    W1g = g1[:, None] * W1
    v1 = b1ln @ W1 + np.asarray(inputs["b1"], f32)
    Wc1g = g2[:, None] * Wc1
    vc = b2ln @ Wc1 + np.asarray(inputs["bc1"], f32)

    def chunked(a, last):
        return np.ascontiguousarray(a.reshape(4, 128, last).transpose(1, 0, 2))

    bf = ml_dtypes.bfloat16
    return {
        "w1g": chunked(W1g, D).astype(bf),
        "v1": np.ascontiguousarray(v1.reshape(4, 128).T),
        "w2s": np.ascontiguousarray(
            np.asarray(inputs["w2"], f32).reshape(4, 128).T).astype(bf),
        "wc1g": chunked(Wc1g, D).astype(bf),
        "vc": np.ascontiguousarray(vc.reshape(4, 128).T),
        "wc2": chunked(Wc2, NCLS).astype(bf),
        "bc2": np.asarray(inputs["bc2"], f32).reshape(1, NCLS),
    }


_CACHE = {}


def _get_compiled():
    if "nc" not in _CACHE:
        nc = bacc.Bacc("TRN2", target_bir_lowering=False, debug=False,
                       enable_asserts=False, num_devices=N_CORES)
        ins = {
            "x": nc.dram_tensor("x", [N_LOC, D], F32, kind="ExternalInput").ap(),
            "w1g": nc.dram_tensor("w1g", [128, 4, D], BF16,
                                  kind="ExternalInput").ap(),
            "v1": nc.dram_tensor("v1", [128, 4], F32, kind="ExternalInput").ap(),
            "w2s": nc.dram_tensor("w2s", [128, 4], BF16,
                                  kind="ExternalInput").ap(),
            "wc1g": nc.dram_tensor("wc1g", [128, 4, D], BF16,
                                   kind="ExternalInput").ap(),
            "vc": nc.dram_tensor("vc", [128, 4], F32, kind="ExternalInput").ap(),
            "wc2": nc.dram_tensor("wc2", [128, 4, NCLS], BF16,
                                  kind="ExternalInput").ap(),
            "bc2": nc.dram_tensor("bc2", [1, NCLS], F32,
                                  kind="ExternalInput").ap(),
        }
        outs = {
            "bag_out": nc.dram_tensor("bag_out", [NB_LOC, D], F32,
                                      kind="ExternalOutput").ap(),
            "logits_out": nc.dram_tensor("logits_out", [NB_LOC, NCLS], F32,
                                         kind="ExternalOutput").ap(),
        }
        with tile.TileContext(nc) as tc:
            build_mil(tc, outs, ins, N_LOC)
        nc.compile()
        _CACHE["nc"] = nc
    return _CACHE["nc"]


def make_in_maps(inputs):
    params = _prep_params(inputs)
    x = np.asarray(inputs["window_embeddings"], np.float32)
    in_maps = []
    for i in range(N_CORES):
        m = dict(params)
        m["x"] = np.ascontiguousarray(x[i * N_LOC:(i + 1) * N_LOC])
        in_maps.append(m)
    return in_maps


def kernel(**inputs):
    bag_slices = np.asarray(inputs["bag_slices"])
    starts = np.arange(N_BAGS, dtype=np.int64) * BAG
    expect = np.stack([starts, starts + BAG], axis=1)
    assert np.array_equal(bag_slices.astype(np.int64), expect), \
        "kernel assumes contiguous bags of 64 windows"

    nc = _get_compiled()
    res = run_bass_kernel_spmd(nc, make_in_maps(inputs),
                               core_ids=list(range(N_CORES)))
    logits = np.concatenate([r["logits_out"] for r in res.results], axis=0)
    bag_tensor = np.concatenate([r["bag_out"] for r in res.results], axis=0)
    return logits, bag_tensor
